# revision 13
# baseline (speedup 1.0000x reference)
"""BGAT attention kernel for Trainium2 (8 NeuronCores, batch-parallel).

Strategy (per core = one batch element):
  score[u,a,k] = (1/8) * sum_d av[k,d] * lrelu(S), S = (U+A+E)[u,a,(k,d)]
  Using lrelu(x) = 0.6x + 0.4|x|:
    score = T1 + sum_pos |S''| - sum_neg |S''|
  where S'' has per-column weights folded with 0.4/8*|av_d| (columns permuted
  so each head's positive-av columns sit in one padded uniform block, negative
  in another), and T1 = linear term via folded projection columns (exact).
  E-term weights ride a K=65 augmented matmul (ones row x U[u] row) so the
  per-user broadcast add is free; the A-term rides an identity matmul into the
  same PSUM accumulation.
  softmax needs no max-subtraction (scores are tiny by construction).
  Message sums commute with the edge projection:
    sum_a alpha*E = (sum_a alpha*edge) @ We   (and same over u)
  so phase 3 is small matmuls over natural-layout edge tiles.
"""

import math
from contextlib import ExitStack

import ml_dtypes
import numpy as np

BF16 = ml_dtypes.bfloat16

# ---- problem sizes (hardcoded from spec) ----
B = 8
FULL_CFG = dict(NU=256, NA=256, ED=64, UD=128, AD=128, H=8, HD=64)
SLOPE = 0.2


def make_cfg(NU, NA, ED, UD, AD, H, HD, av, UC=None):
    """Host-side layout metadata derived from av sign pattern."""
    cfg = dict(NU=NU, NA=NA, ED=ED, UD=UD, AD=AD, H=H, HD=HD)
    cfg["HH"] = H * HD
    scale = 1.0 / math.sqrt(HD)
    av = np.asarray(av, np.float32)
    pos_idx = [np.nonzero(av[k] >= 0)[0] for k in range(H)]
    neg_idx = [np.nonzero(av[k] < 0)[0] for k in range(H)]
    P_ = max(len(ix) for ix in pos_idx)
    N_ = max(len(ix) for ix in neg_idx)
    cfg["P_"], cfg["N_"] = P_, N_
    cfg["EXTC"] = H * P_ + H * N_ + H
    cfg["pos_idx"], cfg["neg_idx"] = pos_idx, neg_idx
    cfg["scale"] = scale
    cfg["NAH"] = (NA + 127) // 128  # number of 128-wide antenna chunks
    cfg["ACH"] = min(128, NA)
    cfg["UC"] = min(128, NU) if UC is None else UC
    cfg["NUC"] = NU // cfg["UC"]  # number of user chunks
    assert NU % 8 == 0
    cfg["NG"] = NU // 8  # softmax groups of 8 users
    return cfg


def prep_weights(Wu, Wa, We, av, Wres, cfg):
    """Build folded/permuted weight blocks. Returns dict of np arrays."""
    H, HD, ED, UD, AD = cfg["H"], cfg["HD"], cfg["ED"], cfg["UD"], cfg["AD"]
    P_, N_, EXTC, HH = cfg["P_"], cfg["N_"], cfg["EXTC"], cfg["HH"]
    scale = cfg["scale"]
    Wu, Wa, We = (np.asarray(x, np.float32) for x in (Wu, Wa, We))
    av = np.asarray(av, np.float32)
    Wres = np.asarray(Wres, np.float32)

    wu_big = np.zeros((UD, EXTC + HH), np.float32)
    wa_big = np.zeros((AD, EXTC + HH), np.float32)
    we_big = np.zeros((ED, EXTC + HH), np.float32)
    for k in range(H):
        for i, d in enumerate(cfg["pos_idx"][k]):
            c = 0.4 * scale * abs(av[k, d])
            col = k * P_ + i
            wu_big[:, col] = Wu[k][:, d] * c
            wa_big[:, col] = Wa[k][:, d] * c
            we_big[:, col] = We[k][:, d] * c
        for i, d in enumerate(cfg["neg_idx"][k]):
            c = 0.4 * scale * abs(av[k, d])
            col = H * P_ + k * N_ + i
            wu_big[:, col] = Wu[k][:, d] * c
            wa_big[:, col] = Wa[k][:, d] * c
            we_big[:, col] = We[k][:, d] * c
        # T1 (linear) columns: W @ (0.6*scale*av_k)
        t1w = 0.6 * scale * av[k]
        col = H * P_ + H * N_ + k
        wu_big[:, col] = Wu[k] @ t1w
        wa_big[:, col] = Wa[k] @ t1w
        we_big[:, col] = We[k] @ t1w
        # raw blocks for message matmuls
        wu_big[:, EXTC + k * HD : EXTC + (k + 1) * HD] = Wu[k]
        wa_big[:, EXTC + k * HD : EXTC + (k + 1) * HD] = Wa[k]
        we_big[:, EXTC + k * HD : EXTC + (k + 1) * HD] = We[k]

    ident = np.eye(128, dtype=np.float32)
    return dict(wu_big=wu_big.astype(BF16), wa_big=wa_big.astype(BF16),
                we_big=we_big.astype(BF16), wres=Wres.astype(BF16),
                ident=ident.astype(BF16))


def build_bgat(ctx: ExitStack, tc, outs, ins, cfg):
    """Emit the Tile program. outs/ins: dicts name->AP."""
    import concourse.bass as bass
    import concourse.mybir as mybir

    nc = tc.nc
    f32 = mybir.dt.float32
    bf16 = mybir.dt.bfloat16
    AX = mybir.AxisListType.X
    ADD = mybir.AluOpType.add
    EXPF = mybir.ActivationFunctionType.Exp

    NU, NA, ED, UD, AD = cfg["NU"], cfg["NA"], cfg["ED"], cfg["UD"], cfg["AD"]
    H, HD, HH = cfg["H"], cfg["HD"], cfg["HH"]
    P_, N_, EXTC = cfg["P_"], cfg["N_"], cfg["EXTC"]
    NAH, ACH, UC, NUC, NG = cfg["NAH"], cfg["ACH"], cfg["UC"], cfg["NUC"], cfg["NG"]
    HIDDEN = HH
    POSW, NEGW = H * P_, H * N_

    edge = ins["edge"]      # [NU*NA, ED]
    user = ins["user"]      # [NU, UD]
    ant = ins["ant"]        # [NA, AD]
    wu_big_d = ins["wu_big"]
    wa_big_d = ins["wa_big"]
    we_big_d = ins["we_big"]
    wres_d = ins["wres"]
    ident_d = ins["ident"]
    user_out = outs["user_out"]  # [NU, HIDDEN]
    ant_out = outs["ant_out"]    # [NA, HIDDEN]

    # x-major chunked view of edge: chunk c has 128 consecutive (u,a) rows
    CH = ACH  # rows per chunk (128 at full size)
    n_chunks_per_u = NAH
    edge_x = edge.rearrange("(c p) e -> c p e", p=CH)
    # u-major view for ant-side: partition = user
    edge_u = edge.rearrange("(j p a) e -> j p (a e)", p=UC, a=NA)

    consts = ctx.enter_context(tc.tile_pool(name="consts", bufs=1))

    # ---------- persistent SBUF tensors ----------
    ident_sb = consts.tile([128, 128], bf16)
    nc.sync.dma_start(ident_sb[:], ident_d[:, :])
    wu_big_sb = consts.tile([UD, EXTC + HH], bf16)
    nc.sync.dma_start(wu_big_sb[:], wu_big_d[:, :])
    wa_big_sb = consts.tile([AD, EXTC + HH], bf16)
    nc.sync.dma_start(wa_big_sb[:], wa_big_d[:, :])
    we_big_sb = consts.tile([ED, EXTC + HH], bf16)
    nc.sync.dma_start(we_big_sb[:], we_big_d[:, :])
    wres_sb = consts.tile([UD, HIDDEN], bf16)
    nc.sync.dma_start(wres_sb[:], wres_d[:, :])

    ones_col = consts.tile([128, 1], bf16)
    nc.gpsimd.memset(ones_col[:], 1.0)
    ones_row = consts.tile([1, 128], bf16)
    nc.gpsimd.memset(ones_row[:], 1.0)

    U_big = consts.tile([UC, NUC, EXTC + HH], bf16)
    A_big = consts.tile([ACH, NAH, EXTC + HH], bf16)
    userT = consts.tile([UD, NU], bf16)
    antT = consts.tile([AD, NA], bf16)
    # alpha layouts: v3 = antenna-major, head-outer; v2 = user-major
    alpha_v3 = consts.tile([ACH, NAH, H, NU], bf16)
    alpha_v2 = consts.tile([UC, NUC, H, NA], bf16)
    ew_all = consts.tile([ED, NU, H], bf16)
    ewa_all = consts.tile([ED, NA, H], bf16)

    # combo rhs tiles (rows 0..ED-1 = we_big ext cols, row ED = per-user U row)
    combo0 = consts.tile([ED + 1, EXTC], bf16)
    combo1 = consts.tile([ED + 1, EXTC], bf16)
    combos = [combo0, combo1]
    for cb in combos:
        nc.gpsimd.dma_start(cb[0:ED, :], we_big_d[:, 0:EXTC])

    # ---------- precompute: transposes and U/A projections ----------
    with tc.tile_pool(name="pre_sb", bufs=2) as pre_sb, \
         tc.tile_pool(name="pre_ps", bufs=2, space="PSUM") as pre_ps:
        # user/ant feature tiles and transposes
        for (feat, T_sb, n, fd) in ((user, userT, NU, UD), (ant, antT, NA, AD)):
            fv = feat.rearrange("(j p) f -> j p f", p=min(128, n))
            for j in range(fv.shape[0]):
                p = fv.shape[1]
                ft = pre_sb.tile([p, fd], bf16, tag="ft")
                nc.sync.dma_start(ft[:], fv[j])
                pt = pre_ps.tile([fd, p], bf16, tag="pt")
                nc.tensor.transpose(pt[:], ft[:], ident_sb[0:p, 0:p])
                nc.scalar.copy(T_sb[:, j * p : j * p + p], pt[:])
        # U_big / A_big
        for (T_sb, big, nchunk, pc, fd) in (
            (userT, U_big, NUC, UC, UD),
            (antT, A_big, NAH, ACH, AD),
        ):
            w_sb = wu_big_sb if big is U_big else wa_big_sb
            for j in range(nchunk):
                for c0 in range(0, EXTC + HH, 512):
                    c1 = min(c0 + 512, EXTC + HH)
                    ps = pre_ps.tile([pc, 512], f32, tag="proj")
                    nc.tensor.matmul(ps[:, 0 : c1 - c0],
                                     T_sb[:, j * pc : j * pc + pc],
                                     w_sb[:, c0:c1], start=True, stop=True)
                    nc.scalar.copy(big[:, j, c0:c1], ps[:, 0 : c1 - c0])

    # ---------- pass 1: scores + softmax + user-side weighted edge sums ----
    # psum_misc bank layout (per group of 8 users):
    T1_OFF = 0                      # [128, NAH*8*H]
    SUM_OFF = T1_OFF + NAH * 8 * H  # [1, 8*H]
    RB_OFF = SUM_OFF + 8 * H        # [128, 8*H]
    EW_OFF = RB_OFF + 8 * H         # [ED, 8*H]
    assert EW_OFF + 8 * H <= 512

    with tc.tile_pool(name="edge_pool", bufs=6 * NAH + 2 * 8 * NAH) as edge_pool, \
         tc.tile_pool(name="p1_sb", bufs=3) as p1_sb, \
         tc.tile_pool(name="p1_stage", bufs=2) as p1_stage, \
         tc.tile_pool(name="ps_pos", bufs=2, space="PSUM") as ps_pos_pool, \
         tc.tile_pool(name="ps_neg", bufs=2, space="PSUM") as ps_neg_pool, \
         tc.tile_pool(name="ps_tp", bufs=2, space="PSUM") as ps_tp_pool, \
         tc.tile_pool(name="ps_misc", bufs=2, space="PSUM") as ps_misc_pool:

        # chunk list per group, in emission order, processed in sub-batches
        # of 4 (4 transposes share one psum bank + one batched copy)
        assert (8 * NAH) % 4 == 0
        for g in range(NG):
            misc = ps_misc_pool.tile([128, 512], f32, tag="misc")
            stage_P = p1_stage.tile([ACH, NAH * 8, H], f32, tag="sP")
            stage_N = p1_stage.tile([ACH, NAH * 8, H], f32, tag="sN")
            edge_tiles = {}
            chunks = [(ui, h) for ui in range(8) for h in range(NAH)]
            for u4 in range(0, len(chunks), 4):
                batch = chunks[u4 : u4 + 4]
                tp = ps_tp_pool.tile([ED, 512], bf16, tag="tp")
                edT4 = p1_sb.tile([ED + 1, 512], bf16, tag="edT4")
                nc.vector.memset(edT4[ED : ED + 1, :], 1.0)
                for q, (ui, h) in enumerate(batch):
                    u = g * 8 + ui
                    c = u * n_chunks_per_u + h
                    et = edge_pool.tile([CH, ED], bf16, tag="edge")
                    nc.sync.dma_start(et[:], edge_x[c])
                    edge_tiles[(ui, h)] = et
                    nc.tensor.transpose(tp[:, q * 128 : q * 128 + CH], et[:],
                                        ident_sb[0:CH, 0:CH])
                for q, (ui, h) in enumerate(batch):
                    if h == 0:
                        u = g * 8 + ui
                        cb = combos[u % 2]
                        # per-user U row into combo row ED
                        nc.gpsimd.dma_start(
                            cb[ED : ED + 1, :],
                            U_big[u % UC : u % UC + 1, u // UC, 0:EXTC])
                nc.scalar.copy(edT4[0:ED, :], tp[:, :])
                for q, (ui, h) in enumerate(batch):
                    u = g * 8 + ui
                    cb = combos[u % 2]
                    sidx = h * 8 + ui
                    t1s = misc[0:CH, T1_OFF + sidx * H : T1_OFF + (sidx + 1) * H]
                    lhs = edT4[0 : ED + 1, q * 128 : q * 128 + CH]
                    ps_pos = ps_pos_pool.tile([CH, 512], f32, tag="pos")
                    ps_neg = ps_neg_pool.tile([CH, 512], f32, tag="neg")
                    # E+U into psum (K=ED+1 augmented), then A via identity mm
                    nc.tensor.matmul(ps_pos[:, 0:POSW], lhs, cb[:, 0:POSW],
                                     start=True, stop=False)
                    nc.tensor.matmul(ps_neg[:, 0:NEGW], lhs,
                                     cb[:, POSW : POSW + NEGW],
                                     start=True, stop=False)
                    nc.tensor.matmul(t1s, lhs, cb[:, POSW + NEGW : EXTC],
                                     start=True, stop=False)
                    nc.tensor.matmul(ps_pos[:, 0:POSW], ident_sb[0:ACH, 0:ACH],
                                     A_big[:, h, 0:POSW], start=False, stop=True)
                    nc.tensor.matmul(ps_neg[:, 0:NEGW], ident_sb[0:ACH, 0:ACH],
                                     A_big[:, h, POSW : POSW + NEGW],
                                     start=False, stop=True)
                    nc.tensor.matmul(t1s, ident_sb[0:ACH, 0:ACH],
                                     A_big[:, h, POSW + NEGW : EXTC],
                                     start=False, stop=True)
                    # |.| reduces
                    nc.vector.tensor_reduce(
                        stage_P[:, sidx, :],
                        ps_pos[:, 0:POSW].rearrange("p (k d) -> p k d", d=P_),
                        axis=AX, op=ADD, apply_absolute_value=True)
                    nc.vector.tensor_reduce(
                        stage_N[:, sidx, :],
                        ps_neg[:, 0:NEGW].rearrange("p (k d) -> p k d", d=N_),
                        axis=AX, op=ADD, apply_absolute_value=True)

            # ---- group softmax ----
            # score_g memory order (h, u, k); exp_g memory order (h, k, u)
            gsz = NAH * 8 * H
            score_g = p1_sb.tile([ACH, gsz], f32, tag="score", bufs=4)
            nc.vector.tensor_sub(score_g[:],
                                 stage_P[:].rearrange("p a b -> p (a b)"),
                                 stage_N[:].rearrange("p a b -> p (a b)"))
            nc.vector.tensor_add(score_g[:], score_g[:],
                                 misc[0:ACH, T1_OFF : T1_OFF + gsz])
            exp_g = p1_sb.tile([ACH, gsz], bf16, tag="expg", bufs=6)
            nc.scalar.activation(
                exp_g[:].rearrange("p (a c b) -> p a b c", a=NAH, c=H),
                score_g[:].rearrange("p (a b c) -> p a b c", a=NAH, b=8),
                EXPF)
            for h in range(NAH):
                nc.tensor.matmul(
                    misc[0:1, SUM_OFF : SUM_OFF + 8 * H], ones_col[0:ACH, :],
                    exp_g[:, h * 8 * H : (h + 1) * 8 * H],
                    start=(h == 0), stop=(h == NAH - 1))
            rec = p1_sb.tile([1, 8 * H], f32, tag="rec", bufs=4)
            nc.vector.reciprocal(rec[:], misc[0:1, SUM_OFF : SUM_OFF + 8 * H])
            rec_bf = p1_sb.tile([1, 8 * H], bf16, tag="recbf", bufs=4)
            nc.vector.tensor_copy(rec_bf[:], rec[:])
            nc.tensor.matmul(misc[0:128, RB_OFF : RB_OFF + 8 * H],
                             ones_row[:, 0:128], rec_bf[:], start=True, stop=True)
            rbs = p1_sb.tile([ACH, 8 * H], bf16, tag="rbs", bufs=4)
            nc.scalar.copy(rbs[:], misc[0:ACH, RB_OFF : RB_OFF + 8 * H])
            # alpha (normalized), kept in flat group tile + scattered to v3
            for h in range(NAH):
                sl = exp_g[:, h * 8 * H : (h + 1) * 8 * H]
                nc.vector.tensor_mul(sl, sl, rbs[:])
                nc.vector.tensor_copy(
                    alpha_v3[:, h, :, g * 8 : g * 8 + 8],
                    sl.rearrange("p (k u) -> p k u", k=H))
            # ---- user-side weighted edge sums ----
            for ui in range(8):
                u = g * 8 + ui
                for h in range(NAH):
                    al_u = exp_g[:, h * 8 * H : (h + 1) * 8 * H].rearrange(
                        "p (k u) -> p k u", k=H)[:, :, ui]
                    nc.tensor.matmul(
                        misc[0:ED, EW_OFF + ui * H : EW_OFF + (ui + 1) * H],
                        edge_tiles[(ui, h)][:], al_u,
                        start=(h == 0), stop=(h == NAH - 1))
            nc.vector.tensor_copy(
                ew_all[:, g * 8 : g * 8 + 8, :].rearrange("p a b -> p (a b)"),
                misc[0:ED, EW_OFF : EW_OFF + 8 * H])

    # ---------- pass 3: ant-side sums and outputs ----------
    with tc.tile_pool(name="p3_sb", bufs=3) as p3_sb, \
         tc.tile_pool(name="p3_ps", bufs=2, space="PSUM") as p3_ps, \
         tc.tile_pool(name="po_ps", bufs=2, space="PSUM") as po_ps:
        # alpha_v2 (user-major) via direct [128,128] transposes of alpha_v3
        for j in range(NUC):
            for k in range(H):
                for h in range(NAH):
                    pt2 = p3_ps.tile([UC, 512], bf16, tag="pt2")
                    nc.tensor.transpose(
                        pt2[:, 0:ACH],
                        alpha_v3[:, h, k, j * UC : (j + 1) * UC],
                        ident_sb[0:ACH, 0:ACH])
                    nc.scalar.copy(
                        alpha_v2[:, j, k, h * ACH : (h + 1) * ACH],
                        pt2[0:UC, 0:ACH])
        # ant-side weighted edge sums (contract over users); edge streamed
        # u-major in 8-antenna slabs
        edge_u4 = edge.rearrange("(j p a) e -> j p a e", p=UC, a=NA)
        for ag in range(NA // 8):
            ev = p3_sb.tile([UC, NUC, 8, ED], bf16, tag="ev")
            for j in range(NUC):
                for ap2 in range(0, 8, 4):
                    nc.sync.dma_start(
                        ev[:, j, ap2 : ap2 + 4, :],
                        edge_u4[j, :, ag * 8 + ap2 : ag * 8 + ap2 + 4, :])
            pe = p3_ps.tile([ED, 512], f32, tag="pewa")
            for ai in range(8):
                a = ag * 8 + ai
                for j in range(NUC):
                    nc.tensor.matmul(
                        pe[:, ai * H : (ai + 1) * H],
                        ev[:, j, ai, :], alpha_v2[:, j, :, a],
                        start=(j == 0), stop=(j == NUC - 1))
            nc.vector.tensor_copy(
                ewa_all[:, ag * 8 : ag * 8 + 8, :].rearrange("p a b -> p (a b)"),
                pe[:, 0 : 8 * H])
        # user_out = concat_k(alpha@A_k + ew@We_k) + user@Wres
        uo_v = user_out.rearrange("(j p) d -> j p d", p=UC)
        for j in range(NUC):
            po = po_ps.tile([UC, HIDDEN], f32, tag="puo")
            for k in range(H):
                nc.tensor.matmul(po[:, k * HD : (k + 1) * HD],
                                 userT[:, j * UC : j * UC + UC],
                                 wres_sb[:, k * HD : (k + 1) * HD],
                                 start=True, stop=False)
                for h in range(NAH):
                    nc.tensor.matmul(
                        po[:, k * HD : (k + 1) * HD],
                        alpha_v3[:, h, k, j * UC : j * UC + UC],
                        A_big[:, h, EXTC + k * HD : EXTC + (k + 1) * HD],
                        start=False, stop=False)
                nc.tensor.matmul(
                    po[:, k * HD : (k + 1) * HD],
                    ew_all[:, j * UC : j * UC + UC, k],
                    we_big_sb[:, EXTC + k * HD : EXTC + (k + 1) * HD],
                    start=False, stop=True)
            ob = p3_sb.tile([UC, HIDDEN], f32, tag="ob")
            nc.scalar.copy(ob[:], po[:])
            nc.sync.dma_start(uo_v[j], ob[:])
        # ant_out = concat_k(alpha^T@U_k + ewa@We_k)
        ao_v = ant_out.rearrange("(i p) d -> i p d", p=ACH)
        for i in range(NA // ACH):
            po = po_ps.tile([ACH, HIDDEN], f32, tag="pao")
            for k in range(H):
                for j in range(NUC):
                    nc.tensor.matmul(
                        po[:, k * HD : (k + 1) * HD],
                        alpha_v2[:, j, k, i * ACH : (i + 1) * ACH],
                        U_big[:, j, EXTC + k * HD : EXTC + (k + 1) * HD],
                        start=(j == 0), stop=False)
                nc.tensor.matmul(
                    po[:, k * HD : (k + 1) * HD],
                    ewa_all[:, i * ACH : (i + 1) * ACH, k],
                    we_big_sb[:, EXTC + k * HD : EXTC + (k + 1) * HD],
                    start=False, stop=True)
            ob = p3_sb.tile([ACH, HIDDEN], f32, tag="ob2")
            nc.scalar.copy(ob[:], po[:])
            nc.sync.dma_start(ao_v[i], ob[:])


# ---------------------------------------------------------------------------
_CACHE = {}


def _get_nc(cfg):
    key = "nc"
    if key in _CACHE:
        return _CACHE[key]
    import concourse.bacc as bacc
    import concourse.mybir as mybir
    import concourse.tile as tile

    f32 = mybir.dt.float32
    bf16 = mybir.dt.bfloat16
    nc = bacc.Bacc("TRN2", target_bir_lowering=False, debug=False)
    NU, NA, ED, UD, AD = cfg["NU"], cfg["NA"], cfg["ED"], cfg["UD"], cfg["AD"]
    EXTC, HH = cfg["EXTC"], cfg["HH"]
    ins = {
        "edge": nc.dram_tensor("edge", [NU * NA, ED], bf16, kind="ExternalInput").ap(),
        "user": nc.dram_tensor("user", [NU, UD], bf16, kind="ExternalInput").ap(),
        "ant": nc.dram_tensor("ant", [NA, AD], bf16, kind="ExternalInput").ap(),
        "wu_big": nc.dram_tensor("wu_big", [UD, EXTC + HH], bf16, kind="ExternalInput").ap(),
        "wa_big": nc.dram_tensor("wa_big", [AD, EXTC + HH], bf16, kind="ExternalInput").ap(),
        "we_big": nc.dram_tensor("we_big", [ED, EXTC + HH], bf16, kind="ExternalInput").ap(),
        "wres": nc.dram_tensor("wres", [UD, HH], bf16, kind="ExternalInput").ap(),
        "ident": nc.dram_tensor("ident", [128, 128], bf16, kind="ExternalInput").ap(),
    }
    outs = {
        "user_out": nc.dram_tensor("user_out", [NU, HH], f32, kind="ExternalOutput").ap(),
        "ant_out": nc.dram_tensor("ant_out", [NA, HH], f32, kind="ExternalOutput").ap(),
    }
    with tile.TileContext(nc) as tc:
        with ExitStack() as ctx:
            build_bgat(ctx, tc, outs, ins, cfg)
    nc.finalize()
    _CACHE[key] = nc
    return nc


_LAST_RES = {}


def kernel(user_feats, ant_feats, edge_feats, Wu, Wa, We, av, Wres,
           _trace=False):
    from concourse.bass_utils import run_bass_kernel_spmd

    user_feats = np.asarray(user_feats, np.float32).astype(BF16)
    ant_feats = np.asarray(ant_feats, np.float32).astype(BF16)
    edge_feats = np.asarray(edge_feats, np.float32).astype(BF16)
    cfg = make_cfg(**FULL_CFG, av=av)
    wd = prep_weights(Wu, Wa, We, av, Wres, cfg)
    nc = _get_nc(cfg)
    NU, NA, ED = cfg["NU"], cfg["NA"], cfg["ED"]
    in_maps = []
    for b in range(B):
        in_maps.append({
            "edge": edge_feats[b].reshape(NU * NA, ED),
            "user": user_feats[b],
            "ant": ant_feats[b],
            "wu_big": wd["wu_big"], "wa_big": wd["wa_big"],
            "we_big": wd["we_big"], "wres": wd["wres"], "ident": wd["ident"],
        })
    res = run_bass_kernel_spmd(nc, in_maps, core_ids=list(range(B)),
                               trace=_trace)
    _LAST_RES["res"] = res
    user_out = np.stack([res.results[b]["user_out"] for b in range(B)])
    ant_out = np.stack([res.results[b]["ant_out"] for b in range(B)])
    return (user_out, ant_out)



# revision 21
# speedup vs baseline: 1.0326x; 1.0326x over previous
"""BGAT attention kernel for Trainium2 (8 NeuronCores, batch-parallel).

Strategy (per core = one batch element):
  score[u,a,k] = (1/8) * sum_d av[k,d] * lrelu(S), S = (U+A+E)[u,a,(k,d)]
  Using lrelu(x) = 0.6x + 0.4|x|:
    score = T1 + sum_pos |S''| - sum_neg |S''|
  where S'' has per-column weights folded with 0.4/8*|av_d| (columns permuted
  so each head's positive-av columns sit in one padded uniform block, negative
  in another), and T1 = linear term via folded projection columns (exact).
  E-term weights ride a K=65 augmented matmul (ones row x U[u] row) so the
  per-user broadcast add is free; the A-term rides an identity matmul into the
  same PSUM accumulation.
  softmax needs no max-subtraction (scores are tiny by construction).
  Message sums commute with the edge projection:
    sum_a alpha*E = (sum_a alpha*edge) @ We   (and same over u)
  so phase 3 is small matmuls over natural-layout edge tiles.
"""

import math
from contextlib import ExitStack

import ml_dtypes
import numpy as np

BF16 = ml_dtypes.bfloat16

# ---- problem sizes (hardcoded from spec) ----
B = 8
FULL_CFG = dict(NU=256, NA=256, ED=64, UD=128, AD=128, H=8, HD=64)
SLOPE = 0.2


def make_cfg(NU, NA, ED, UD, AD, H, HD, av, UC=None):
    """Host-side layout metadata derived from av sign pattern.

    Per-head flip: block A holds the smaller of (pos, neg) index sets so
    padding is minimal; sigma[k] = +1 if A=pos else -1; heads are ordered
    so unflipped (sigma=+1) heads come first (head_order[r] = orig head).
    score_k = T1_k + sigma_k * (sumA_k - sumB_k).
    """
    cfg = dict(NU=NU, NA=NA, ED=ED, UD=UD, AD=AD, H=H, HD=HD)
    cfg["HH"] = H * HD
    scale = 1.0 / math.sqrt(HD)
    av = np.asarray(av, np.float32)
    pos_idx = [np.nonzero(av[k] >= 0)[0] for k in range(H)]
    neg_idx = [np.nonzero(av[k] < 0)[0] for k in range(H)]
    flip = [len(pos_idx[k]) > len(neg_idx[k]) for k in range(H)]
    A_idx = [neg_idx[k] if flip[k] else pos_idx[k] for k in range(H)]
    B_idx = [pos_idx[k] if flip[k] else neg_idx[k] for k in range(H)]
    order = sorted(range(H), key=lambda k: flip[k])  # unflipped first
    m_unflipped = sum(1 for k in order if not flip[k])
    A_ = max(len(ix) for ix in A_idx)
    B_ = max(len(ix) for ix in B_idx)
    cfg["A_"], cfg["B_"] = A_, B_
    cfg["AW"], cfg["BW"] = H * A_, H * B_
    cfg["EXTC"] = H * A_ + H * B_ + H
    cfg["A_idx"], cfg["B_idx"] = A_idx, B_idx
    cfg["head_order"] = order
    cfg["m_unflipped"] = m_unflipped
    cfg["scale"] = scale
    cfg["NAH"] = (NA + 127) // 128  # number of 128-wide antenna chunks
    cfg["ACH"] = min(128, NA)
    cfg["UC"] = min(128, NU) if UC is None else UC
    cfg["NUC"] = NU // cfg["UC"]  # number of user chunks
    assert NU % 8 == 0
    cfg["NG"] = NU // 8  # softmax groups of 8 users
    return cfg


def prep_weights(Wu, Wa, We, av, Wres, cfg):
    """Build folded/permuted weight blocks. Returns dict of np arrays."""
    H, HD, ED, UD, AD = cfg["H"], cfg["HD"], cfg["ED"], cfg["UD"], cfg["AD"]
    A_, B_, EXTC, HH = cfg["A_"], cfg["B_"], cfg["EXTC"], cfg["HH"]
    AW = cfg["AW"]
    scale = cfg["scale"]
    order = cfg["head_order"]
    Wu, Wa, We = (np.asarray(x, np.float32) for x in (Wu, Wa, We))
    av = np.asarray(av, np.float32)
    Wres = np.asarray(Wres, np.float32)

    wu_big = np.zeros((UD, EXTC + HH), np.float32)
    wa_big = np.zeros((AD, EXTC + HH), np.float32)
    we_big = np.zeros((ED, EXTC + HH), np.float32)
    for r, k in enumerate(order):
        for i, d in enumerate(cfg["A_idx"][k]):
            c = 0.4 * scale * abs(av[k, d])
            col = r * A_ + i
            wu_big[:, col] = Wu[k][:, d] * c
            wa_big[:, col] = Wa[k][:, d] * c
            we_big[:, col] = We[k][:, d] * c
        for i, d in enumerate(cfg["B_idx"][k]):
            c = 0.4 * scale * abs(av[k, d])
            col = AW + r * B_ + i
            wu_big[:, col] = Wu[k][:, d] * c
            wa_big[:, col] = Wa[k][:, d] * c
            we_big[:, col] = We[k][:, d] * c
        # T1 (linear) columns: W @ (0.6*scale*av_k), in score-head order
        t1w = 0.6 * scale * av[k]
        col = AW + H * B_ + r
        wu_big[:, col] = Wu[k] @ t1w
        wa_big[:, col] = Wa[k] @ t1w
        we_big[:, col] = We[k] @ t1w
    for k in range(H):
        # raw blocks for message matmuls (original head order)
        wu_big[:, EXTC + k * HD : EXTC + (k + 1) * HD] = Wu[k]
        wa_big[:, EXTC + k * HD : EXTC + (k + 1) * HD] = Wa[k]
        we_big[:, EXTC + k * HD : EXTC + (k + 1) * HD] = We[k]

    ident = np.eye(128, dtype=np.float32)
    return dict(wu_big=wu_big.astype(BF16), wa_big=wa_big.astype(BF16),
                we_big=we_big.astype(BF16), wres=Wres.astype(BF16),
                ident=ident.astype(BF16))


def build_bgat(ctx: ExitStack, tc, outs, ins, cfg):
    """Emit the Tile program. outs/ins: dicts name->AP."""
    import concourse.bass as bass
    import concourse.mybir as mybir

    nc = tc.nc
    f32 = mybir.dt.float32
    bf16 = mybir.dt.bfloat16
    AX = mybir.AxisListType.X
    ADD = mybir.AluOpType.add
    EXPF = mybir.ActivationFunctionType.Exp

    NU, NA, ED, UD, AD = cfg["NU"], cfg["NA"], cfg["ED"], cfg["UD"], cfg["AD"]
    H, HD, HH = cfg["H"], cfg["HD"], cfg["HH"]
    A_, B_, EXTC = cfg["A_"], cfg["B_"], cfg["EXTC"]
    NAH, ACH, UC, NUC, NG = cfg["NAH"], cfg["ACH"], cfg["UC"], cfg["NUC"], cfg["NG"]
    HIDDEN = HH
    AW, BW = cfg["AW"], cfg["BW"]
    M_UNF = cfg["m_unflipped"]
    rank_of = [0] * H
    for r, k in enumerate(cfg["head_order"]):
        rank_of[k] = r

    edge = ins["edge"]      # [NU*NA, ED]
    user = ins["user"]      # [NU, UD]
    ant = ins["ant"]        # [NA, AD]
    wu_big_d = ins["wu_big"]
    wa_big_d = ins["wa_big"]
    we_big_d = ins["we_big"]
    wres_d = ins["wres"]
    ident_d = ins["ident"]
    user_out = outs["user_out"]  # [NU, HIDDEN]
    ant_out = outs["ant_out"]    # [NA, HIDDEN]

    # x-major chunked view of edge: chunk c has 128 consecutive (u,a) rows
    CH = ACH  # rows per chunk (128 at full size)
    n_chunks_per_u = NAH
    edge_x = edge.rearrange("(c p) e -> c p e", p=CH)
    # u-major view for ant-side: partition = user
    edge_u = edge.rearrange("(j p a) e -> j p (a e)", p=UC, a=NA)

    consts = ctx.enter_context(tc.tile_pool(name="consts", bufs=1))

    # ---------- persistent SBUF tensors ----------
    ident_sb = consts.tile([128, 128], bf16)
    nc.sync.dma_start(ident_sb[:], ident_d[:, :])
    wu_big_sb = consts.tile([UD, EXTC + HH], bf16)
    nc.sync.dma_start(wu_big_sb[:], wu_big_d[:, :])
    wa_big_sb = consts.tile([AD, EXTC + HH], bf16)
    nc.sync.dma_start(wa_big_sb[:], wa_big_d[:, :])
    we_big_sb = consts.tile([ED, EXTC + HH], bf16)
    nc.sync.dma_start(we_big_sb[:], we_big_d[:, :])
    wres_sb = consts.tile([UD, HIDDEN], bf16)
    nc.sync.dma_start(wres_sb[:], wres_d[:, :])

    ones_col = consts.tile([128, 1], bf16)
    nc.gpsimd.memset(ones_col[:], 1.0)
    ones_row = consts.tile([1, 128], bf16)
    nc.gpsimd.memset(ones_row[:], 1.0)

    U_big = consts.tile([UC, NUC, EXTC + HH], bf16)
    A_big = consts.tile([ACH, NAH, EXTC + HH], bf16)
    userT = consts.tile([UD, NU], bf16)
    antT = consts.tile([AD, NA], bf16)
    # alpha layouts: v3 = antenna-major, head-outer; v2 = user-major
    alpha_v3 = consts.tile([ACH, NAH, H, NU], bf16)
    alpha_v2 = consts.tile([UC, NUC, H, NA], bf16)
    ew_all = consts.tile([ED, NU, H], bf16)
    ewa_all = consts.tile([ED, NA, H], bf16)

    # combo rhs tiles (rows 0..ED-1 = we_big ext cols, row ED = per-user U row)
    combo0 = consts.tile([ED + 1, EXTC], bf16)
    combo1 = consts.tile([ED + 1, EXTC], bf16)
    combos = [combo0, combo1]
    for cb in combos:
        nc.gpsimd.dma_start(cb[0:ED, :], we_big_d[:, 0:EXTC])

    # ---------- precompute: transposes and U/A projections ----------
    with tc.tile_pool(name="pre_sb", bufs=2) as pre_sb, \
         tc.tile_pool(name="pre_ps", bufs=2, space="PSUM") as pre_ps:
        # user/ant feature tiles and transposes
        for (feat, T_sb, n, fd) in ((user, userT, NU, UD), (ant, antT, NA, AD)):
            fv = feat.rearrange("(j p) f -> j p f", p=min(128, n))
            for j in range(fv.shape[0]):
                p = fv.shape[1]
                ft = pre_sb.tile([p, fd], bf16, tag="ft")
                nc.sync.dma_start(ft[:], fv[j])
                pt = pre_ps.tile([fd, p], bf16, tag="pt")
                nc.tensor.transpose(pt[:], ft[:], ident_sb[0:p, 0:p])
                nc.scalar.copy(T_sb[:, j * p : j * p + p], pt[:])
        # U_big / A_big
        for (T_sb, big, nchunk, pc, fd) in (
            (userT, U_big, NUC, UC, UD),
            (antT, A_big, NAH, ACH, AD),
        ):
            w_sb = wu_big_sb if big is U_big else wa_big_sb
            for j in range(nchunk):
                for c0 in range(0, EXTC + HH, 512):
                    c1 = min(c0 + 512, EXTC + HH)
                    ps = pre_ps.tile([pc, 512], f32, tag="proj")
                    nc.tensor.matmul(ps[:, 0 : c1 - c0],
                                     T_sb[:, j * pc : j * pc + pc],
                                     w_sb[:, c0:c1], start=True, stop=True)
                    nc.scalar.copy(big[:, j, c0:c1], ps[:, 0 : c1 - c0])

    # ---------- pass 1: scores + softmax + user-side weighted edge sums ----
    # psum_misc bank layout (per group of 8 users):
    T1_OFF = 0                      # [128, NAH*8*H]
    SUM_OFF = T1_OFF + NAH * 8 * H  # [1, 8*H]
    RB_OFF = SUM_OFF + 8 * H        # [128, 8*H]
    EW_OFF = RB_OFF + 8 * H         # [ED, 8*H]
    assert EW_OFF + 8 * H <= 512

    with tc.tile_pool(name="edge_pool", bufs=10) as edge_pool, \
         tc.tile_pool(name="p1_sb", bufs=3) as p1_sb, \
         tc.tile_pool(name="p1_stage", bufs=2) as p1_stage, \
         tc.tile_pool(name="ps_pos", bufs=2, space="PSUM") as ps_pos_pool, \
         tc.tile_pool(name="ps_neg", bufs=2, space="PSUM") as ps_neg_pool, \
         tc.tile_pool(name="ps_tp", bufs=2, space="PSUM") as ps_tp_pool, \
         tc.tile_pool(name="ps_misc", bufs=2, space="PSUM") as ps_misc_pool:

        # chunk list per group, in emission order, processed in sub-batches
        # of 4 (4 transposes share one psum bank + one batched copy + 1 DMA)
        assert (8 * NAH) % 4 == 0
        edge_x4 = edge.rearrange("(cc c p) e -> cc p c e", c=4, p=CH)
        for g in range(NG):
            misc = ps_misc_pool.tile([128, 512], f32, tag="misc")
            stage_A = p1_stage.tile([ACH, NAH * 8, H], f32, tag="sA")
            stage_B = p1_stage.tile([ACH, NAH * 8, H], f32, tag="sB")
            edge_tiles = {}
            chunks = [(ui, h) for ui in range(8) for h in range(NAH)]
            for u4 in range(0, len(chunks), 4):
                batch = chunks[u4 : u4 + 4]
                tp = ps_tp_pool.tile([ED, 512], bf16, tag="tp")
                edT4 = p1_sb.tile([ED + 1, 512], bf16, tag="edT4")
                nc.gpsimd.memset(edT4[ED : ED + 1, :], 1.0)
                # one DMA covers the whole 4-chunk batch (contiguous in HBM)
                c0 = (g * 8 + batch[0][0]) * n_chunks_per_u + batch[0][1]
                assert c0 % 4 == 0
                et4 = edge_pool.tile([CH, 4, ED], bf16, tag="edge")
                nc.sync.dma_start(et4[:], edge_x4[c0 // 4])
                for q, (ui, h) in enumerate(batch):
                    edge_tiles[(ui, h)] = et4[:, q, :]
                    nc.tensor.transpose(tp[:, q * 128 : q * 128 + CH],
                                        et4[:, q, :], ident_sb[0:CH, 0:CH])
                for q, (ui, h) in enumerate(batch):
                    if h == 0:
                        u = g * 8 + ui
                        cb = combos[u % 2]
                        # per-user U row into combo row ED
                        nc.gpsimd.dma_start(
                            cb[ED : ED + 1, :],
                            U_big[u % UC : u % UC + 1, u // UC, 0:EXTC])
                nc.scalar.copy(edT4[0:ED, :], tp[:, :])
                for q, (ui, h) in enumerate(batch):
                    u = g * 8 + ui
                    cb = combos[u % 2]
                    sidx = h * 8 + ui
                    t1s = misc[0:CH, T1_OFF + sidx * H : T1_OFF + (sidx + 1) * H]
                    lhs = edT4[0 : ED + 1, q * 128 : q * 128 + CH]
                    ps_a = ps_pos_pool.tile([CH, 512], f32, tag="pos")
                    ps_b = ps_neg_pool.tile([CH, 512], f32, tag="neg")
                    # E+U into psum (K=ED+1 augmented), then A via identity mm
                    nc.tensor.matmul(ps_a[:, 0:AW], lhs, cb[:, 0:AW],
                                     start=True, stop=False)
                    nc.tensor.matmul(ps_b[:, 0:BW], lhs,
                                     cb[:, AW : AW + BW],
                                     start=True, stop=False)
                    nc.tensor.matmul(t1s, lhs, cb[:, AW + BW : EXTC],
                                     start=True, stop=False)
                    nc.tensor.matmul(ps_a[:, 0:AW], ident_sb[0:ACH, 0:ACH],
                                     A_big[:, h, 0:AW], start=False, stop=True)
                    nc.tensor.matmul(ps_b[:, 0:BW], ident_sb[0:ACH, 0:ACH],
                                     A_big[:, h, AW : AW + BW],
                                     start=False, stop=True)
                    nc.tensor.matmul(t1s, ident_sb[0:ACH, 0:ACH],
                                     A_big[:, h, AW + BW : EXTC],
                                     start=False, stop=True)
                    # |.| reduces
                    nc.vector.tensor_reduce(
                        stage_A[:, sidx, :],
                        ps_a[:, 0:AW].rearrange("p (k d) -> p k d", d=A_),
                        axis=AX, op=ADD, apply_absolute_value=True)
                    nc.vector.tensor_reduce(
                        stage_B[:, sidx, :],
                        ps_b[:, 0:BW].rearrange("p (k d) -> p k d", d=B_),
                        axis=AX, op=ADD, apply_absolute_value=True)

            # ---- group softmax ----
            # score_g memory order (h, u, k); exp_g memory order (h, k, u)
            gsz = NAH * 8 * H
            score_g = p1_sb.tile([ACH, gsz], f32, tag="score", bufs=4)
            score_g3 = score_g[:].rearrange("p (a b) -> p a b", b=H)
            if M_UNF > 0:
                nc.vector.tensor_sub(score_g3[:, :, 0:M_UNF],
                                     stage_A[:, :, 0:M_UNF],
                                     stage_B[:, :, 0:M_UNF])
            if M_UNF < H:
                nc.vector.tensor_sub(score_g3[:, :, M_UNF:H],
                                     stage_B[:, :, M_UNF:H],
                                     stage_A[:, :, M_UNF:H])
            nc.vector.tensor_add(score_g[:], score_g[:],
                                 misc[0:ACH, T1_OFF : T1_OFF + gsz])
            exp_g = p1_sb.tile([ACH, gsz], bf16, tag="expg", bufs=6)
            nc.scalar.activation(
                exp_g[:].rearrange("p (a c b) -> p a b c", a=NAH, c=H),
                score_g[:].rearrange("p (a b c) -> p a b c", a=NAH, b=8),
                EXPF)
            for h in range(NAH):
                nc.tensor.matmul(
                    misc[0:1, SUM_OFF : SUM_OFF + 8 * H], ones_col[0:ACH, :],
                    exp_g[:, h * 8 * H : (h + 1) * 8 * H],
                    start=(h == 0), stop=(h == NAH - 1))
            rec = p1_sb.tile([1, 8 * H], f32, tag="rec", bufs=4)
            nc.vector.reciprocal(rec[:], misc[0:1, SUM_OFF : SUM_OFF + 8 * H])
            rec_bf = p1_sb.tile([1, 8 * H], bf16, tag="recbf", bufs=4)
            nc.vector.tensor_copy(rec_bf[:], rec[:])
            nc.tensor.matmul(misc[0:128, RB_OFF : RB_OFF + 8 * H],
                             ones_row[:, 0:128], rec_bf[:], start=True, stop=True)
            rbs = p1_sb.tile([ACH, 8 * H], bf16, tag="rbs", bufs=4)
            nc.scalar.copy(rbs[:], misc[0:ACH, RB_OFF : RB_OFF + 8 * H])
            # alpha (normalized), kept in flat group tile + scattered to v3
            for h in range(NAH):
                sl = exp_g[:, h * 8 * H : (h + 1) * 8 * H]
                nc.vector.tensor_mul(sl, sl, rbs[:])
                nc.vector.tensor_copy(
                    alpha_v3[:, h, :, g * 8 : g * 8 + 8],
                    sl.rearrange("p (k u) -> p k u", k=H))
            # ---- user-side weighted edge sums ----
            for ui in range(8):
                u = g * 8 + ui
                for h in range(NAH):
                    al_u = exp_g[:, h * 8 * H : (h + 1) * 8 * H].rearrange(
                        "p (k u) -> p k u", k=H)[:, :, ui]
                    nc.tensor.matmul(
                        misc[0:ED, EW_OFF + ui * H : EW_OFF + (ui + 1) * H],
                        edge_tiles[(ui, h)][:], al_u,
                        start=(h == 0), stop=(h == NAH - 1))
            nc.vector.tensor_copy(
                ew_all[:, g * 8 : g * 8 + 8, :].rearrange("p a b -> p (a b)"),
                misc[0:ED, EW_OFF : EW_OFF + 8 * H])

    # ---------- pass 3: ant-side sums and outputs ----------
    with tc.tile_pool(name="p3_sb", bufs=3) as p3_sb, \
         tc.tile_pool(name="p3_ps", bufs=2, space="PSUM") as p3_ps, \
         tc.tile_pool(name="po_ps", bufs=2, space="PSUM") as po_ps:
        # alpha_v2 (user-major) via direct [128,128] transposes of alpha_v3
        for j in range(NUC):
            for k in range(H):
                for h in range(NAH):
                    pt2 = p3_ps.tile([UC, 512], bf16, tag="pt2")
                    nc.tensor.transpose(
                        pt2[:, 0:ACH],
                        alpha_v3[:, h, k, j * UC : (j + 1) * UC],
                        ident_sb[0:ACH, 0:ACH])
                    nc.scalar.copy(
                        alpha_v2[:, j, k, h * ACH : (h + 1) * ACH],
                        pt2[0:UC, 0:ACH])
        # ant-side weighted edge sums (contract over users); edge streamed
        # u-major in 8-antenna slabs
        edge_u4 = edge.rearrange("(j p a) e -> j p a e", p=UC, a=NA)
        AG = 16  # antennas per slab: 2KB/partition DMA, one psum bank of sums
        for ag in range(NA // AG):
            ev = p3_sb.tile([UC, NUC, AG, ED], bf16, tag="ev")
            for j in range(NUC):
                nc.sync.dma_start(
                    ev[:, j, :, :],
                    edge_u4[j, :, ag * AG : (ag + 1) * AG, :])
            pe = p3_ps.tile([ED, 512], f32, tag="pewa")
            for ai in range(AG):
                a = ag * AG + ai
                for j in range(NUC):
                    nc.tensor.matmul(
                        pe[:, ai * H : (ai + 1) * H],
                        ev[:, j, ai, :], alpha_v2[:, j, :, a],
                        start=(j == 0), stop=(j == NUC - 1))
            nc.scalar.copy(
                ewa_all[:, ag * AG : (ag + 1) * AG, :].rearrange("p a b -> p (a b)"),
                pe[:, 0 : AG * H])
        # user_out = concat_k(alpha@A_k + ew@We_k) + user@Wres
        uo_v = user_out.rearrange("(j p) d -> j p d", p=UC)
        for j in range(NUC):
            po = po_ps.tile([UC, HIDDEN], f32, tag="puo")
            for k in range(H):
                nc.tensor.matmul(po[:, k * HD : (k + 1) * HD],
                                 userT[:, j * UC : j * UC + UC],
                                 wres_sb[:, k * HD : (k + 1) * HD],
                                 start=True, stop=False)
                for h in range(NAH):
                    nc.tensor.matmul(
                        po[:, k * HD : (k + 1) * HD],
                        alpha_v3[:, h, rank_of[k], j * UC : j * UC + UC],
                        A_big[:, h, EXTC + k * HD : EXTC + (k + 1) * HD],
                        start=False, stop=False)
                nc.tensor.matmul(
                    po[:, k * HD : (k + 1) * HD],
                    ew_all[:, j * UC : j * UC + UC, rank_of[k]],
                    we_big_sb[:, EXTC + k * HD : EXTC + (k + 1) * HD],
                    start=False, stop=True)
            ob = p3_sb.tile([UC, HIDDEN], f32, tag="ob")
            nc.scalar.copy(ob[:], po[:])
            nc.sync.dma_start(uo_v[j], ob[:])
        # ant_out = concat_k(alpha^T@U_k + ewa@We_k)
        ao_v = ant_out.rearrange("(i p) d -> i p d", p=ACH)
        for i in range(NA // ACH):
            po = po_ps.tile([ACH, HIDDEN], f32, tag="pao")
            for k in range(H):
                for j in range(NUC):
                    nc.tensor.matmul(
                        po[:, k * HD : (k + 1) * HD],
                        alpha_v2[:, j, rank_of[k], i * ACH : (i + 1) * ACH],
                        U_big[:, j, EXTC + k * HD : EXTC + (k + 1) * HD],
                        start=(j == 0), stop=False)
                nc.tensor.matmul(
                    po[:, k * HD : (k + 1) * HD],
                    ewa_all[:, i * ACH : (i + 1) * ACH, rank_of[k]],
                    we_big_sb[:, EXTC + k * HD : EXTC + (k + 1) * HD],
                    start=False, stop=True)
            ob = p3_sb.tile([ACH, HIDDEN], f32, tag="ob2")
            nc.scalar.copy(ob[:], po[:])
            nc.sync.dma_start(ao_v[i], ob[:])


# ---------------------------------------------------------------------------
_CACHE = {}


def _get_nc(cfg):
    key = "nc"
    if key in _CACHE:
        return _CACHE[key]
    import concourse.bacc as bacc
    import concourse.mybir as mybir
    import concourse.tile as tile

    f32 = mybir.dt.float32
    bf16 = mybir.dt.bfloat16
    nc = bacc.Bacc("TRN2", target_bir_lowering=False, debug=False)
    NU, NA, ED, UD, AD = cfg["NU"], cfg["NA"], cfg["ED"], cfg["UD"], cfg["AD"]
    EXTC, HH = cfg["EXTC"], cfg["HH"]
    ins = {
        "edge": nc.dram_tensor("edge", [NU * NA, ED], bf16, kind="ExternalInput").ap(),
        "user": nc.dram_tensor("user", [NU, UD], bf16, kind="ExternalInput").ap(),
        "ant": nc.dram_tensor("ant", [NA, AD], bf16, kind="ExternalInput").ap(),
        "wu_big": nc.dram_tensor("wu_big", [UD, EXTC + HH], bf16, kind="ExternalInput").ap(),
        "wa_big": nc.dram_tensor("wa_big", [AD, EXTC + HH], bf16, kind="ExternalInput").ap(),
        "we_big": nc.dram_tensor("we_big", [ED, EXTC + HH], bf16, kind="ExternalInput").ap(),
        "wres": nc.dram_tensor("wres", [UD, HH], bf16, kind="ExternalInput").ap(),
        "ident": nc.dram_tensor("ident", [128, 128], bf16, kind="ExternalInput").ap(),
    }
    outs = {
        "user_out": nc.dram_tensor("user_out", [NU, HH], f32, kind="ExternalOutput").ap(),
        "ant_out": nc.dram_tensor("ant_out", [NA, HH], f32, kind="ExternalOutput").ap(),
    }
    with tile.TileContext(nc) as tc:
        with ExitStack() as ctx:
            build_bgat(ctx, tc, outs, ins, cfg)
    nc.finalize()
    _CACHE[key] = nc
    return nc


_LAST_RES = {}


def kernel(user_feats, ant_feats, edge_feats, Wu, Wa, We, av, Wres,
           _trace=False):
    from concourse.bass_utils import run_bass_kernel_spmd

    user_feats = np.asarray(user_feats, np.float32).astype(BF16)
    ant_feats = np.asarray(ant_feats, np.float32).astype(BF16)
    edge_feats = np.asarray(edge_feats, np.float32).astype(BF16)
    cfg = make_cfg(**FULL_CFG, av=av)
    wd = prep_weights(Wu, Wa, We, av, Wres, cfg)
    nc = _get_nc(cfg)
    NU, NA, ED = cfg["NU"], cfg["NA"], cfg["ED"]
    in_maps = []
    for b in range(B):
        in_maps.append({
            "edge": edge_feats[b].reshape(NU * NA, ED),
            "user": user_feats[b],
            "ant": ant_feats[b],
            "wu_big": wd["wu_big"], "wa_big": wd["wa_big"],
            "we_big": wd["we_big"], "wres": wd["wres"], "ident": wd["ident"],
        })
    res = run_bass_kernel_spmd(nc, in_maps, core_ids=list(range(B)),
                               trace=_trace)
    _LAST_RES["res"] = res
    user_out = np.stack([res.results[b]["user_out"] for b in range(B)])
    ant_out = np.stack([res.results[b]["ant_out"] for b in range(B)])
    return (user_out, ant_out)

